# revision 12
# baseline (speedup 1.0000x reference)
"""GAT (2-layer, heads=1) + edge-predictor MLP on 8 Trainium2 NeuronCores.

v2: fp16 end-to-end, batched dma_gather row gathers, fused phases.
  - Nodes sharded 8-way by contiguous dst range; edges bucketed by
    (dst tile, src-half) [src-half needed because dma_gather idxs are i16].
  - Node tables (h1, h2, PS) stored as [*, 256] fp16, AllGathered, and
    row-gathered per edge chunk with one dma_gather per (tile, half).
  - Per-edge src score s = h_src . a_src computed on-device from the
    gathered rows (fused DVE dot); dst score d via per-tile transpose +
    select; exp(lrelu) batched per (tile, half) as exp(max(e, 0.2e)).
  - Softmax-weighted aggregation via one-hot fp16 matmuls; features and
    denominator accumulate in SEPARATE PSUM tiles (interleaved groups in
    one PSUM tile are broken).
  - The SAME edge-slot structure serves L1 agg, L2 agg and the edge
    predictor (PS gathered by src; PD kept SBUF-resident, permuted onto
    edge lanes via one-hot matmul; relu+dot fused in one DVE op).
"""
import sys
sys.path.insert(0, '/opt/trn_rl_repo')
import numpy as np

P = 128
NCORES = 8
NEG = 0.2
DUMMY = 999.0
HALF = 32768
GRP = 8  # idx wrap factor (i16 idxs wrapped over 16 partitions, x8 cores)


def _cfg(N, D, C):
    SH = N // NCORES
    TPS = (SH + P - 1) // P
    SHP = TPS * P
    NFULL = NCORES * SHP
    return dict(N=N, D=D, C=C, SH=SH, TPS=TPS, SHP=SHP, NFULL=NFULL)


def _bucket_edges(src, dst, cfg):
    """Bucket edges by (core, dst tile, src half).

    Returns per-stream (lo/hi) structures shared across cores:
      kt[h][t]   chunk count for tile t in stream h
      idx[h]     [NCORES, 128, TC_h*GRP] i16 wrapped+replicated gather idxs
      sdl[h]     [NCORES, 128, TC_h] f32 dst lanes (DUMMY padding)
      emap[h]    [NCORES, TC_h*128] i64 edge ids (-1 padding)
    """
    SH, TPS, SHP = cfg['SH'], cfg['TPS'], cfg['SHP']
    E = src.shape[0]
    srcrow = (src // SH) * SHP + (src % SH)
    core = dst // SH
    dl = dst - core * SH
    tl = dl // P
    lane = dl % P
    half = (srcrow >= HALF).astype(np.int64)
    key = (core * TPS + tl) * 2 + half
    order = np.argsort(key, kind='stable')
    counts = np.bincount(key, minlength=NCORES * TPS * 2)
    counts = counts.reshape(NCORES, TPS, 2)
    kt = [(counts[:, :, h].max(axis=0) + P - 1) // P for h in (0, 1)]
    csum = np.concatenate([[0], np.cumsum(counts.reshape(-1))])
    eids = np.arange(E, dtype=np.int64)

    out = {}
    for h in (0, 1):
        TC = int(kt[h].sum())
        tile_off = np.concatenate([[0], np.cumsum(kt[h])[:-1]])
        idx_flat = np.zeros((NCORES, TC * P), np.int16)
        sdl = np.full((NCORES, TC * P), DUMMY, np.float32)
        emap = np.full((NCORES, TC * P), -1, np.int64)
        for c in range(NCORES):
            for t in range(TPS):
                k = c * TPS * 2 + t * 2 + h
                n = counts[c, t, h]
                if n == 0:
                    continue
                sl = order[csum[k]:csum[k] + n]
                base = tile_off[t] * P
                idx_flat[c, base:base + n] = (srcrow[sl] - h * HALF).astype(np.int16)
                sdl[c, base:base + n] = lane[sl]
                emap[c, base:base + n] = eids[sl]
        # idxs: element i at [i % 16, i // 16], replicated x8 over partitions
        idxw = np.tile(idx_flat.reshape(NCORES, TC * GRP, 16).transpose(0, 2, 1),
                       (1, 8, 1)).copy()
        # prebuilt predictor one-hots: oht[l, c*128+q] = (sdl_slot[c*128+q]==l)
        oht = np.zeros((NCORES, P, TC * P), np.float16)
        for c in range(NCORES):
            lane_f = sdl[c]  # [TC*P] flat slot-major
            lv = lane_f.astype(np.int64)
            valid = lv < P
            oht[c, lv[valid], np.nonzero(valid)[0]] = 1.0
        # sdl: slot (p, chunk) layout = flat position c*128+p
        sdl = sdl.reshape(NCORES, TC, P).transpose(0, 2, 1).copy()
        out[h] = dict(kt=kt[h], idx=idxw, sdl=sdl, emap=emap, TC=TC, oht=oht)
    return out


def _build(cfg, kt_lo, kt_hi, upto=9, sub=9):
    import concourse.bass as bass
    import concourse.tile as tile
    from concourse import bacc, mybir
    f32 = mybir.dt.float32
    f16 = mybir.dt.float16
    i16 = mybir.dt.int16
    AF = mybir.ActivationFunctionType
    OP = mybir.AluOpType

    D, C = cfg['D'], cfg['C']
    TPS, SHP, NFULL = cfg['TPS'], cfg['SHP'], cfg['NFULL']
    RW = C + 2  # phase-A psum row: [h(256) | s | d]
    KD = D // P
    TClo, TChi = int(kt_lo.sum()), int(kt_hi.sum())
    TC = TClo + TChi
    KTM = int(max(kt_lo.max(), kt_hi.max()))

    nc = bacc.Bacc("TRN2", target_bir_lowering=False, debug=False,
                   num_devices=NCORES, num_swdge_queues=4)

    xT = nc.dram_tensor("xT", [D, SHP], f16, kind="ExternalInput")
    w1s_in = nc.dram_tensor("w1s", [P, KD * RW], f16, kind="ExternalInput")
    w2s_in = nc.dram_tensor("w2s", [P, 2 * RW], f16, kind="ExternalInput")
    wpt_in = nc.dram_tensor("wpt_s", [P, 2 * C], f16, kind="ExternalInput")
    wpb_in = nc.dram_tensor("wpb_s", [P, 2 * C], f16, kind="ExternalInput")
    bv2_in = nc.dram_tensor("bv2", [1, RW], f16, kind="ExternalInput")
    bvs_in = nc.dram_tensor("bvs", [1, C], f16, kind="ExternalInput")
    bvd_in = nc.dram_tensor("bvd", [1, C], f16, kind="ExternalInput")
    a1_in = nc.dram_tensor("a1r", [P, C], f16, kind="ExternalInput")
    a2_in = nc.dram_tensor("a2r", [P, C], f16, kind="ExternalInput")
    wp2_in = nc.dram_tensor("wp2r", [P, C], f16, kind="ExternalInput")
    iota_t_in = nc.dram_tensor("iota_t", [P, P], f16, kind="ExternalInput")
    iota_c_in = nc.dram_tensor("iota_c", [P, 1], f32, kind="ExternalInput")
    ident_in = nc.dram_tensor("ident", [P, P], f32, kind="ExternalInput")
    identh_in = nc.dram_tensor("identh", [P, P], f16, kind="ExternalInput")
    idxlo_in = nc.dram_tensor("idx_lo", [P, max(TClo, 1) * GRP], i16,
                              kind="ExternalInput")
    idxhi_in = nc.dram_tensor("idx_hi", [P, max(TChi, 1) * GRP], i16,
                              kind="ExternalInput")
    sdllo_in = nc.dram_tensor("sdl_lo", [P, max(TClo, 1)], f32,
                              kind="ExternalInput")
    sdlhi_in = nc.dram_tensor("sdl_hi", [P, max(TChi, 1)], f32,
                              kind="ExternalInput")
    ohtlo_in = nc.dram_tensor("oht_lo", [P, max(TClo, 1) * P], f16,
                              kind="ExternalInput")
    ohthi_in = nc.dram_tensor("oht_hi", [P, max(TChi, 1) * P], f16,
                              kind="ExternalInput")

    outp = nc.dram_tensor("outp", [P, TC], f32, kind="ExternalOutput")

    with tile.TileContext(nc) as tc:
        with tc.tile_pool(name="const", bufs=1) as cst, \
             tc.tile_pool(name="own", bufs=1) as own, \
             tc.tile_pool(name="wk", bufs=1) as wk, \
             tc.tile_pool(name="gat", bufs=1) as gat, \
             tc.tile_pool(name="psF", bufs=1, space="PSUM") as psF, \
             tc.tile_pool(name="psD", bufs=1, space="PSUM") as psD, \
             tc.tile_pool(name="psT", bufs=1, space="PSUM") as psT, \
             tc.tile_pool(name="psM", bufs=1, space="PSUM") as psM, \
             tc.tile_pool(name="dram", bufs=1, space="DRAM") as dram:

            # ---------------- constants ----------------
            def ld(name, shape, dt, src):
                t = cst.tile(shape, dt, name=name)
                nc.sync.dma_start(out=t[:], in_=src.ap()[:, :])
                return t

            w1s = ld("w1s_t", [P, KD * RW], f16, w1s_in)
            w2s = ld("w2s_t", [P, 2 * RW], f16, w2s_in)
            wpt_s = ld("wpt_t", [P, 2 * C], f16, wpt_in)
            wpb_s = ld("wpb_t", [P, 2 * C], f16, wpb_in)
            bv2 = ld("bv2_t", [1, RW], f16, bv2_in)
            bvs = ld("bvs_t", [1, C], f16, bvs_in)
            bvd = ld("bvd_t", [1, C], f16, bvd_in)
            a1r = ld("a1_t", [P, C], f16, a1_in)
            a2r = ld("a2_t", [P, C], f16, a2_in)
            wp2r = ld("wp2_t", [P, C], f16, wp2_in)
            iota_t = ld("iot", [P, P], f16, iota_t_in)
            iota_c = ld("ioc", [P, 1], f32, iota_c_in)
            ident = ld("idf", [P, P], f32, ident_in)
            identh = ld("idh", [P, P], f16, identh_in)
            idx_lo = ld("ixl", [P, max(TClo, 1) * GRP], i16, idxlo_in)
            idx_hi = ld("ixh", [P, max(TChi, 1) * GRP], i16, idxhi_in)
            sdl_lo = ld("sdl", [P, max(TClo, 1)], f32, sdllo_in)
            sdl_hi = ld("sdh", [P, max(TChi, 1)], f32, sdlhi_in)
            ones_row = cst.tile([1, P], f16, name="ones_row")
            nc.vector.memset(ones_row[:], 1.0)
            ones_col = cst.tile([P, 1], f16, name="ones_col")
            nc.vector.memset(ones_col[:], 1.0)

            h1_own = cst.tile([P, TPS * C], f16, name="h1_own")
            h2_own = cst.tile([P, TPS * C], f16, name="h2_own")
            pd_all = cst.tile([P, TPS * C], f16, name="pd_all")
            s1o = cst.tile([P, TPS], f32, name="s1o")
            d1o = cst.tile([P, TPS], f32, name="d1o")
            s2o = cst.tile([P, TPS], f32, name="s2o")
            d2o = cst.tile([P, TPS], f32, name="d2o")
            outp_sb = cst.tile([P, TC], f32, name="outp_sb")
            nc.vector.memset(outp_sb[:], 0.0)

            # ---------------- DRAM scratch ----------------
            bounce1 = dram.tile([SHP, C], f16, name="bounce1")
            hf1 = dram.tile([NFULL, C], f16, addr_space="Shared", name="hf1")
            bounce2 = dram.tile([SHP, C], f16, name="bounce2")
            hf2 = dram.tile([NFULL, C], f16, addr_space="Shared", name="hf2")
            psl = dram.tile([SHP, C], f16, name="psl")
            psf = dram.tile([NFULL, C], f16, addr_space="Shared", name="psf")

            from concourse import mybir as _mb

            streams = [
                dict(kt=kt_lo, idx=idx_lo, sdl=sdl_lo, lo=0, hi=HALF,
                     TC=TClo, oht=ohtlo_in),
                dict(kt=kt_hi, idx=idx_hi, sdl=sdl_hi, lo=HALF, hi=NFULL,
                     TC=TChi, oht=ohthi_in),
            ]
            # column offset of tile t within each stream
            for st in streams:
                st['off'] = np.concatenate([[0], np.cumsum(st['kt'])[:-1]])

            # ============ phase A: h1/s1/d1 shard ============
            for t in range(TPS):
                xt = wk.tile([P, KD * P], f16, tag="xt", bufs=2, name="xt")
                nc.sync.dma_start(
                    out=xt[:].rearrange("p (k i) -> p k i", k=KD),
                    in_=xT.ap().rearrange("(k p) i -> p k i", p=P)[:, :, t * P:(t + 1) * P])
                ps = psM.tile([P, RW], f32, tag="psA", bufs=2, name="psA")
                for kc in range(KD):
                    nc.tensor.matmul(ps[:], lhsT=xt[:, kc * P:(kc + 1) * P],
                                     rhs=w1s[:, kc * RW:(kc + 1) * RW],
                                     start=(kc == 0), stop=False)
                nc.tensor.matmul(ps[:], lhsT=ones_row[:1, :], rhs=bv2[:1, :RW],
                                 start=False, stop=True)
                nc.scalar.activation(h1_own[:, t * C:(t + 1) * C], ps[:, :C],
                                     AF.Copy)
                nc.vector.tensor_copy(s1o[:, t:t + 1], ps[:, C:C + 1])
                nc.vector.tensor_copy(d1o[:, t:t + 1], ps[:, C + 1:C + 2])
                nc.sync.dma_start(out=bounce1[t * P:(t + 1) * P, :],
                                  in_=h1_own[:, t * C:(t + 1) * C])

            nc.gpsimd.collective_compute(
                "AllGather", _mb.AluOpType.bypass,
                replica_groups=[list(range(NCORES))],
                ins=[bounce1[:, :]], outs=[hf1[:, :]])

            def self_ex(so, do, name):
                e = wk.tile([P, TPS], f32, name=name + "_e")
                nc.vector.tensor_add(e[:], so[:], do[:])
                u = wk.tile([P, TPS], f32, name=name + "_u")
                nc.vector.tensor_scalar_mul(u[:], e[:], NEG)
                z = wk.tile([P, TPS], f32, name=name + "_z")
                nc.vector.tensor_tensor(out=z[:], in0=e[:], in1=u[:], op=OP.max)
                ex = wk.tile([P, TPS], f32, name=name + "_ex")
                nc.scalar.activation(ex[:], z[:], AF.Exp)
                return ex

            # ============ aggregation layer ============
            GG = 8  # max chunks per dma_gather (1024 idxs: HW ucode limit)
            qrr = [0]  # round-robin over the 4 SWDGE queues

            def make_fetcher(st, hf, tag):
                state = dict(groups={})

                def fetch(ci):
                    gi = ci // GG
                    if gi not in state['groups']:
                        n = min(GG, st['TC'] - gi * GG)
                        g = gat.tile([P, GG * C], f16, tag=tag, bufs=5,
                                     name="g")
                        nc.gpsimd.dma_gather(
                            out_ap=g[:, :n * C].rearrange(
                                "p (k r) -> p k r", k=n),
                            in_ap=hf[st['lo']:st['hi'], :],
                            idxs_ap=st['idx'][:, gi * GG * GRP:
                                              (gi * GG + n) * GRP],
                            num_idxs=n * P, num_idxs_reg=n * P, elem_size=C,
                            queue_num=qrr[0])
                        qrr[0] = (qrr[0] + 1) % 4
                        state['groups'][gi] = g  # issued once, in order
                    return state['groups'][gi], ci - gi * GG
                return fetch

            def agg_layer(hf, h_own, ar, do, ex_self, finalize):
                for st, tg in zip(streams, ("glo", "ghi")):
                    st['fetch'] = make_fetcher(st, hf, tg)
                for t in range(TPS):
                    # dst-score row for this tile, broadcast via transpose
                    pT = psT.tile([P, P], f32, tag="pT", bufs=2, name="pT")
                    nc.tensor.transpose(
                        out=pT[:], in_=do[:, t:t + 1].to_broadcast([P, P]),
                        identity=ident[:])
                    dsel = wk.tile([P, P], f16, tag="dsel", bufs=2, name="dsel")
                    nc.vector.tensor_copy(dsel[:], pT[:])

                    ps_f = psF.tile([P, C], f32, tag="psf", bufs=2, name="ps_f")
                    ps_d = psD.tile([P, 1], f32, tag="psd", bufs=2, name="ps_d")
                    # self chunk
                    ohs = wk.tile([P, P], f16, tag="ohw", bufs=3, name="ohs")
                    nc.vector.tensor_scalar(out=ohs[:], in0=iota_t[:],
                                            scalar1=iota_c[:, :1],
                                            scalar2=ex_self[:, t:t + 1],
                                            op0=OP.is_equal, op1=OP.mult)
                    nchunks = int(streams[0]['kt'][t] + streams[1]['kt'][t])
                    if sub >= 3:
                        nc.tensor.matmul(ps_f[:], lhsT=ohs[:],
                                         rhs=h_own[:, t * C:(t + 1) * C],
                                         start=True, stop=(nchunks == 0))
                        nc.tensor.matmul(ps_d[:], lhsT=ohs[:], rhs=ones_col[:],
                                         start=True, stop=(nchunks == 0))
                    done = 0
                    for st in streams:
                        kt = int(st['kt'][t])
                        if kt == 0:
                            continue
                        c0 = int(st['off'][t])
                        sE = wk.tile([P, KTM], f32, tag="sE", bufs=2, name="sE")
                        dE = wk.tile([P, KTM], f32, tag="dE", bufs=2, name="dE")
                        for j in range(kt):
                            gt, go = st['fetch'](c0 + j)
                            nc.vector.scalar_tensor_tensor(
                                out=wk.tile([P, C], f16, tag="jk", bufs=2,
                                            name="jk")[:],
                                in0=gt[:, go * C:(go + 1) * C],
                                scalar=1.0, in1=ar[:],
                                op0=OP.mult, op1=OP.mult,
                                accum_out=sE[:, j:j + 1])
                            if sub >= 2:
                                nc.vector.scalar_tensor_tensor(
                                    out=wk.tile([P, P], f16, tag="jk2", bufs=2,
                                                name="junk2")[:],
                                    in0=iota_t[:],
                                    scalar=st['sdl'][:, c0 + j:c0 + j + 1],
                                    in1=dsel[:], op0=OP.is_equal, op1=OP.mult,
                                    accum_out=dE[:, j:j + 1])
                        if sub < 2:
                            continue
                        eT = wk.tile([P, KTM], f32, tag="eT", bufs=2, name="eT")
                        nc.vector.tensor_add(eT[:, :kt], sE[:, :kt], dE[:, :kt])
                        uT = wk.tile([P, KTM], f32, tag="uT", bufs=2, name="uT")
                        nc.vector.tensor_scalar_mul(uT[:, :kt], eT[:, :kt], NEG)
                        zT = wk.tile([P, KTM], f32, tag="zT", bufs=2, name="zT")
                        nc.vector.tensor_tensor(out=zT[:, :kt], in0=eT[:, :kt],
                                                in1=uT[:, :kt], op=OP.max)
                        exT = wk.tile([P, KTM], f32, tag="exT", bufs=2, name="exT")
                        nc.scalar.activation(exT[:, :kt], zT[:, :kt], AF.Exp)
                        for j in range(kt):
                            gt, go = st['fetch'](c0 + j)
                            ohw = wk.tile([P, P], f16, tag="ohw", bufs=3,
                                          name="ohw")
                            nc.vector.tensor_scalar(
                                out=ohw[:], in0=iota_t[:],
                                scalar1=st['sdl'][:, c0 + j:c0 + j + 1],
                                scalar2=exT[:, j:j + 1],
                                op0=OP.is_equal, op1=OP.mult)
                            done += 1
                            last = (done == nchunks)
                            if sub >= 3:
                                nc.tensor.matmul(ps_f[:], lhsT=ohw[:],
                                                 rhs=gt[:, go * C:(go + 1) * C],
                                                 start=False, stop=last)
                                nc.tensor.matmul(ps_d[:], lhsT=ohw[:],
                                                 rhs=ones_col[:],
                                                 start=False, stop=last)
                    # normalize
                    if sub < 4:
                        continue
                    rcp = wk.tile([P, 1], f32, tag="rcp", bufs=2, name="rcp")
                    nc.vector.reciprocal(rcp[:], ps_d[:, :1])
                    o = wk.tile([P, C], f32, tag="onrm", bufs=2, name="o")
                    nc.scalar.activation(o[:], ps_f[:], AF.Copy, scale=rcp[:, :1])
                    # transpose halves into SBUF for downstream matmuls
                    oTs = []
                    for half in (0, 1):
                        pX = psT.tile([P, P], f32, tag="pT", bufs=2, name="pX")
                        nc.tensor.transpose(out=pX[:],
                                            in_=o[:, half * P:(half + 1) * P],
                                            identity=ident[:])
                        oT = wk.tile([P, P], f16, tag="oT", bufs=4, name="oT")
                        nc.vector.tensor_copy(oT[:], pX[:])
                        oTs.append(oT)
                    finalize(t, oTs)

            # ---- L1 finalize: h2 = out1 @ W2e + bv2 ----
            def fin1(t, oTs):
                ps2 = psM.tile([P, RW], f32, tag="psA", bufs=2, name="ps2")
                nc.tensor.matmul(ps2[:], lhsT=oTs[0][:], rhs=w2s[:, :RW],
                                 start=True, stop=False)
                nc.tensor.matmul(ps2[:], lhsT=oTs[1][:], rhs=w2s[:, RW:2 * RW],
                                 start=False, stop=False)
                nc.tensor.matmul(ps2[:], lhsT=ones_row[:1, :], rhs=bv2[:1, :RW],
                                 start=False, stop=True)
                nc.scalar.activation(h2_own[:, t * C:(t + 1) * C], ps2[:, :C],
                                     AF.Copy)
                nc.vector.tensor_copy(s2o[:, t:t + 1], ps2[:, C:C + 1])
                nc.vector.tensor_copy(d2o[:, t:t + 1], ps2[:, C + 1:C + 2])
                nc.sync.dma_start(out=bounce2[t * P:(t + 1) * P, :],
                                  in_=h2_own[:, t * C:(t + 1) * C])

            # ---- L2 finalize: PD (SBUF) and PS (-> psl) ----
            def fin2(t, oTs):
                pd = psM.tile([P, RW], f32, tag="psA", bufs=2, name="pdp")
                nc.tensor.matmul(pd[:, :C], lhsT=oTs[0][:], rhs=wpb_s[:, :C],
                                 start=True, stop=False)
                nc.tensor.matmul(pd[:, :C], lhsT=oTs[1][:], rhs=wpb_s[:, C:2 * C],
                                 start=False, stop=False)
                nc.tensor.matmul(pd[:, :C], lhsT=ones_row[:1, :], rhs=bvd[:1, :],
                                 start=False, stop=True)
                nc.scalar.activation(pd_all[:, t * C:(t + 1) * C], pd[:, :C],
                                     AF.Copy)
                pp = psM.tile([P, RW], f32, tag="psA", bufs=2, name="ppp")
                nc.tensor.matmul(pp[:, :C], lhsT=oTs[0][:], rhs=wpt_s[:, :C],
                                 start=True, stop=False)
                nc.tensor.matmul(pp[:, :C], lhsT=oTs[1][:], rhs=wpt_s[:, C:2 * C],
                                 start=False, stop=False)
                nc.tensor.matmul(pp[:, :C], lhsT=ones_row[:1, :], rhs=bvs[:1, :],
                                 start=False, stop=True)
                pst = wk.tile([P, C], f16, tag="pst", bufs=2, name="pst")
                nc.scalar.activation(pst[:], pp[:, :C], AF.Copy)
                nc.sync.dma_start(out=psl[t * P:(t + 1) * P, :], in_=pst[:])

            ex1 = self_ex(s1o, d1o, "x1")
            if upto >= 2:
                agg_layer(hf1, h1_own, a1r, d1o, ex1, fin1)

            if upto >= 3:
                nc.gpsimd.collective_compute(
                    "AllGather", _mb.AluOpType.bypass,
                    replica_groups=[list(range(NCORES))],
                    ins=[bounce2[:, :]], outs=[hf2[:, :]])

            if upto >= 4:
                ex2 = self_ex(s2o, d2o, "x2")
                agg_layer(hf2, h2_own, a2r, d2o, ex2, fin2)

            if upto >= 5:
                nc.gpsimd.collective_compute(
                    "AllGather", _mb.AluOpType.bypass,
                    replica_groups=[list(range(NCORES))],
                    ins=[psl[:, :]], outs=[psf[:, :]])

            # ============ predictor ============
            if upto >= 6:
                for st, tg in zip(streams, ("glo", "ghi")):
                    st['fetch'] = make_fetcher(st, psf, tg)
            for t in range(TPS if upto >= 6 else 0):
                for si, st in enumerate(streams):
                    kt = int(st['kt'][t])
                    if kt == 0:
                        continue
                    c0 = int(st['off'][t])
                    for j in range(kt):
                        gp, gpo = st['fetch'](c0 + j)
                        ohT = wk.tile([P, P], f16, tag="ohT", bufs=6, name="ohT")
                        nc.sync.dma_start(
                            out=ohT[:],
                            in_=st['oht'].ap()[:, (c0 + j) * P:(c0 + j + 1) * P])
                        pz = psM.tile([P, RW], f32, tag="psA", bufs=2, name="pz")
                        nc.tensor.matmul(pz[:, :C], lhsT=ohT[:],
                                         rhs=pd_all[:, t * C:(t + 1) * C],
                                         start=True, stop=False)
                        nc.tensor.matmul(pz[:, :C], lhsT=identh[:],
                                         rhs=gp[:, gpo * C:(gpo + 1) * C],
                                         start=False, stop=True)
                        junk3 = wk.tile([P, C], f32, tag="jk3", bufs=2,
                                        name="junk3")
                        col = (0 if si == 0 else TClo) + c0 + j
                        nc.vector.scalar_tensor_tensor(
                            out=junk3[:], in0=pz[:, :C], scalar=0.0, in1=wp2r[:],
                            op0=OP.max, op1=OP.mult,
                            accum_out=outp_sb[:, col:col + 1])
            nc.sync.dma_start(out=outp.ap()[:, :], in_=outp_sb[:])

    nc.compile()
    return nc


def _prep_inputs(x, edge_index, W1, a_src1, a_dst1, b1, W2, a_src2, a_dst2,
                 b2, Wp1, bp1, wp2, bp2, cfg):
    N, D, C = cfg['N'], cfg['D'], cfg['C']
    SH, TPS, SHP = cfg['SH'], cfg['TPS'], cfg['SHP']
    RW = C + 2
    KD = D // P

    src = edge_index[0].astype(np.int64)
    dst = edge_index[1].astype(np.int64)
    bk = _bucket_edges(src, dst, cfg)

    def ext(W, a_s, a_d):
        w = np.zeros((W.shape[0], RW), np.float32)
        w[:, :C] = W
        w[:, C] = W @ a_s
        w[:, C + 1] = W @ a_d
        return w

    w1e = ext(W1, a_src1, a_dst1)
    w2e = ext(W2, a_src2, a_dst2)
    bv2 = (b1 @ w2e)[None, :].astype(np.float16)
    wpt = Wp1[:C, :].astype(np.float32)
    wpb = Wp1[C:, :].astype(np.float32)
    bvs = (b2 @ wpt)[None, :].astype(np.float16)
    bvd = ((b2 @ wpb) + bp1)[None, :].astype(np.float16)

    # host-side [p, k, :] weight layouts
    w1s = w1e.reshape(KD, P, RW).transpose(1, 0, 2).reshape(P, KD * RW)
    w2s = w2e.reshape(2, P, RW).transpose(1, 0, 2).reshape(P, 2 * RW)
    wpt_s = wpt.reshape(2, P, C).transpose(1, 0, 2).reshape(P, 2 * C)
    wpb_s = wpb.reshape(2, P, C).transpose(1, 0, 2).reshape(P, 2 * C)

    iota_t = np.tile(np.arange(P, dtype=np.float16)[None, :], (P, 1))
    iota_c = np.arange(P, dtype=np.float32)[:, None]
    ident = np.eye(P, dtype=np.float32)
    identh = np.eye(P, dtype=np.float16)
    a1r = np.tile(a_src1[None, :], (P, 1)).astype(np.float16)
    a2r = np.tile(a_src2[None, :], (P, 1)).astype(np.float16)
    wp2r = np.tile(wp2[None, :], (P, 1)).astype(np.float16)

    xf = np.asarray(x, np.float32)
    in_maps = []
    for c in range(NCORES):
        xTc = np.zeros((D, SHP), np.float16)
        xTc[:, :SH] = xf[c * SH:(c + 1) * SH].T
        in_maps.append({
            "xT": xTc,
            "w1s": w1s.astype(np.float16), "w2s": w2s.astype(np.float16),
            "wpt_s": wpt_s.astype(np.float16), "wpb_s": wpb_s.astype(np.float16),
            "bv2": bv2, "bvs": bvs, "bvd": bvd,
            "a1r": a1r, "a2r": a2r, "wp2r": wp2r,
            "iota_t": iota_t, "iota_c": iota_c, "ident": ident, "identh": identh,
            "idx_lo": bk[0]['idx'][c], "idx_hi": bk[1]['idx'][c],
            "sdl_lo": bk[0]['sdl'][c], "sdl_hi": bk[1]['sdl'][c],
            "oht_lo": bk[0]['oht'][c], "oht_hi": bk[1]['oht'][c],
        })
    return in_maps, bk


def _run(inputs, cfg, trace=False):
    from concourse import bass_utils
    in_maps, bk = _prep_inputs(cfg=cfg, **inputs)
    nc = _build(cfg, bk[0]['kt'], bk[1]['kt'], upto=cfg.get('upto', 9),
                sub=cfg.get('sub', 9))
    res = bass_utils.run_bass_kernel_spmd(nc, in_maps,
                                          core_ids=list(range(NCORES)),
                                          trace=trace)
    E = inputs['edge_index'].shape[1]
    TClo = bk[0]['TC']
    out = np.zeros(E, np.float32)
    for c in range(NCORES):
        flat = res.results[c]["outp"].T.reshape(-1)  # [TC*128] chunk-major
        for h, coff in ((0, 0), (1, TClo * P)):
            m = bk[h]['emap'][c]
            valid = m >= 0
            out[m[valid]] = flat[coff:coff + m.shape[0]][valid]
    out += np.float32(inputs['bp2'])
    return out, res


def kernel(**inputs):
    inputs = {k: np.asarray(v) for k, v in inputs.items()}
    cfg = _cfg(N=50000, D=2048, C=256)
    out, _ = _run(inputs, cfg)
    return out
